# revision 1
# baseline (speedup 1.0000x reference)
"""Trainium2 Bass kernel for a (quirky) transformer decoder layer.

Problem shapes: B=2, S=2048, D=128, H=8 heads, head_dim=16.
  sa  = attn(q=x_tgt, kv=x_tgt);  r1 = sa @ w1 + b1 + x_tgt
  ca  = attn(q=enc_out, kv=x_tgt); r2 = ca @ w2 + b2 + r1
  ln  = (r2 - mean) / var   (var unbiased, divide by var not std)
  out = relu(ln @ w3 + b3) @ w4 + b4 + r2
(mask_src / mask_tgt are unused by the reference.)

Sharding: 8 cores, query-row sharding (zero communication). Core c handles
batch c//4, query rows [(c%4)*512 : (c%4+1)*512]. K/V are computed per-core
from the full 2048-row x_tgt of its batch (small replicated work).

On-chip layout: activations kept transposed [d, q] so weights are stationary
matmul operands. Scores are computed transposed (s^T[k, q]) via the fused
projection G_h = wk_h^T @ Q_h^T so that s^T = x @ G_h contracts over the full
128 input channels. Softmax skips max-subtraction (scores bounded ~|3| after
the 1/4 scale), exp runs on ScalarE reading 4 PSUM banks per instruction
(free dim 2048 = 4 heads x 512 queries), the denominator comes from an
all-ones column in a head-packed V (pv writes 4 heads into one PSUM bank via
32-column tile_position groups), and normalization broadcasts the reciprocal
denominator across partitions with a selector matmul.
"""

import numpy as np

import concourse.bass as bass
import concourse.tile as tile
from concourse import mybir
from concourse.bass_utils import run_bass_kernel_spmd

B, S, D, H, HD = 2, 2048, 128, 8, 16
QC = 512  # query rows per core
NCORES = 8
KT = 16  # number of 128-row key tiles
F32 = mybir.dt.float32
F32R = mybir.dt.float32r
AF = mybir.ActivationFunctionType
OP = mybir.AluOpType


# ---------------------------------------------------------------- host packing
def _pack32_cols(w, grp):
    """[D, 128]: col 32g+j (j<16) = w[:, j*H + (4*grp+g)], else 0.

    Used for wq (query projection producing 32-row-padded Q^T) and wv
    (value projection producing the head-packed V; col 32g+16 stays 0 and is
    later memset to 1 on device for the softmax denominator)."""
    out = np.zeros((D, 128), np.float32)
    for g in range(4):
        h = 4 * grp + g
        for j in range(HD):
            out[:, 32 * g + j] = w[:, j * H + h]
    return out


def _perm_head_major(w):
    """[D, D]: col 16h+j = w[:, j*H+h] (head-major column order)."""
    idx = np.empty(D, np.int64)
    for h in range(H):
        for j in range(HD):
            idx[16 * h + j] = j * H + h
    return np.ascontiguousarray(w[:, idx])


def _wk_head_T(w):
    """[16, H*D]: row j, cols 128h:128h+128 = wk[:, j*H+h]."""
    out = np.zeros((HD, H * D), np.float32)
    for h in range(H):
        for j in range(HD):
            out[j, 128 * h : 128 * (h + 1)] = w[:, j * H + h]
    return out


def _pack_w12(w, grp):
    """lhsT for the merge projection: row 32c+j = w[j*H + (4*grp+c), :]."""
    out = np.zeros((D, D), np.float32)
    for c in range(4):
        h = 4 * grp + c
        for j in range(HD):
            out[32 * c + j, :] = w[j * H + h, :]
    return out


def _shuf(a):
    """[T*128, 128] -> [128, T*128]: natural 128-row tiles along free dim."""
    t = a.shape[0] // 128
    return np.ascontiguousarray(
        a.reshape(t, 128, 128).transpose(1, 0, 2).reshape(128, t * 128)
    )


def _unshuf(y):
    """[128, 512] -> [512, 128]"""
    return y.reshape(128, 4, 128).transpose(1, 0, 2).reshape(512, 128)


def _sel_matrix():
    sel = np.zeros((128, 128), np.float32)
    for m in range(128):
        sel[32 * (m // 32) + 16, m] = 1.0
    return sel


def _split_multiwaits(nc):
    """Post-pass for walrus builds that accept only ONE sync-wait per
    instruction: split every instruction carrying N>1 waits into (N-1)
    single-wait NOPs on the same engine placed immediately before it."""
    uid = 0
    for f in nc.m.functions:
        for bb in f.blocks:
            il = bb.instructions
            if not any(
                i.sync_info is not None
                and i.sync_info.on_wait
                and len(i.sync_info.on_wait) > 1
                for i in il
            ):
                continue
            out = []
            for inst in il:
                si = inst.sync_info
                if si is not None and si.on_wait and len(si.on_wait) > 1:
                    waits = list(si.on_wait)
                    for w in waits[:-1]:
                        uid += 1
                        nop = mybir.InstNoOp(
                            name=f"WSPLIT-{uid}",
                            engine=inst.engine,
                            ins=[],
                            outs=[],
                            sync_info=mybir.SyncInfo(on_wait=[w], on_update=[]),
                        )
                        out.append(nop)
                    inst.sync_info = mybir.SyncInfo(
                        on_wait=[waits[-1]], on_update=list(si.on_update)
                    )
                out.append(inst)
            bb.instructions = out
    return nc


# ---------------------------------------------------------------- device build
def build_nc():
    nc = bass.Bass()

    def din(name, shape, dt=F32):
        return nc.dram_tensor(name, list(shape), dt, kind="ExternalInput")

    xb = din("xb", (128, 2048))  # batch x_tgt, 128-row tiles along free dim
    xq = din("xq", (128, 512))  # this core's x_tgt query slice
    eo = din("eo", (128, 512))  # this core's enc_out query slice
    wqh = [din(f"wqh{a}", (D, D), F32R) for a in range(2)]  # head-major cols
    wkh = [din(f"wkh{a}", (HD, H * D), F32R) for a in range(2)]  # wk_h^T stack
    wv_st = din("wv_st", (D, 512), F32R)  # [v_selfA | v_selfB | v_crossA | v_crossB]
    w1p = [din(f"w1p{g}", (D, D), F32R) for g in range(2)]
    w2p = [din(f"w2p{g}", (D, D), F32R) for g in range(2)]
    w3 = din("w3", (D, 512), F32R)
    w4r = din("w4r", (128, 512), F32R)  # col block j = w4[128j:128j+128, :]
    ones_v = din("ones_v", (128, 256), F32R)  # V-aug denominator columns
    selt = din("selt", (128, 128))  # SEL[p, m] = (p == 32*(m//32)+16)
    ident = din("ident", (128, 128))
    ones_col = din("ones_col", (128, 1))
    ones_row = din("ones_row", (1, 128))
    b1t = din("b1t", (128, 1))
    b2t = din("b2t", (128, 1))
    b3t = din("b3t", (128, 4))
    b4t = din("b4t", (128, 1))
    y = nc.dram_tensor("y", [128, 512], F32, kind="ExternalOutput")

    with tile.TileContext(nc) as tc:
        with tc.tile_pool(name="persist", bufs=1) as pp:

            def sbuf(name, shape, dt=F32):
                return pp.tile(list(shape), dt, name=name, tag=name)

            def load(name, dram, shape, dt=F32):
                t = sbuf(name, shape, dt)
                nc.sync.dma_start(out=t[:], in_=dram[:])
                return t

            # ---- constant / weight loads
            wq_t = [load(f"wq{a}", wqh[a], (D, D), F32R) for a in range(2)]
            wk_t = [load(f"wk{a}", wkh[a], (HD, H * D), F32R) for a in range(2)]
            wv_t = load("wv", wv_st, (D, 512), F32R)
            w1p_t = [load(f"w1p{g}", w1p[g], (D, D), F32R) for g in range(2)]
            w2p_t = [load(f"w2p{g}", w2p[g], (D, D), F32R) for g in range(2)]
            w3_t = load("w3", w3, (D, 512), F32R)
            w4_t = load("w4", w4r, (128, 512), F32R)
            sel_t = load("sel", selt, (128, 128))
            id_t = load("id", ident, (128, 128))
            onec_t = load("onec", ones_col, (128, 1))
            oner_t = load("oner", ones_row, (1, 128))
            b1_t = load("b1", b1t, (128, 1))
            b2_t = load("b2", b2t, (128, 1))
            b3_t = load("b3", b3t, (128, 4))
            b4_t = load("b4", b4t, (128, 1))

            xb_t = load("xbn", xb, (128, 2048))
            xq_t = load("xqn", xq, (128, 512))
            eo_t = load("eon", eo, (128, 512))

            xbT = sbuf("xbT", (128, 2048), F32R)
            xqT = sbuf("xqT", (128, 512), F32R)
            eoT = sbuf("eoT", (128, 512), F32R)
            v_all = sbuf("v_all", (128, 16, 512), F32R)
            g_s = [sbuf(f"gs{h}", (128, 512), F32R) for h in range(H)]
            g_c = [sbuf(f"gc{h}", (128, 512), F32R) for h in range(H)]
            qth = [[sbuf(f"qh{a}{h}", (HD, 512), F32R) for h in range(H)]
                   for a in range(2)]

            # ---------------- setup phase: transposes + projections
            with tc.tile_pool(name="pset", bufs=2, space="PSUM") as pset:

                def transpose_into(dst_ap, src_ap, name):
                    ps = pset.tile([128, 128], F32, name=name, tag="tps")
                    nc.tensor.transpose(ps[:], src_ap, id_t[:])
                    nc.vector.tensor_copy(out=dst_ap, in_=ps[:])

                for j in range(4):
                    transpose_into(xqT[:, 128 * j : 128 * (j + 1)],
                                   xq_t[:, 128 * j : 128 * (j + 1)], f"trq{j}")
                for j in range(4):
                    transpose_into(eoT[:, 128 * j : 128 * (j + 1)],
                                   eo_t[:, 128 * j : 128 * (j + 1)], f"tre{j}")
                for j in range(16):
                    transpose_into(xbT[:, 128 * j : 128 * (j + 1)],
                                   xb_t[:, 128 * j : 128 * (j + 1)], f"trb{j}")

                # per-head Q^T: out [16, 512] = wq_hm[:, 16h:16h+16].T @ x^T
                for a, xsrc in ((0, xqT), (1, eoT)):
                    for h in range(H):
                        qps = pset.tile([HD, 512], F32, name=f"qps{a}{h}",
                                        tag="qps")
                        nc.tensor.matmul(
                            qps[:], lhsT=wq_t[a][:, 16 * h : 16 * (h + 1)],
                            rhs=xsrc[:], start=True, stop=True)
                        nc.scalar.copy(out=qth[a][h][:], in_=qps[:])

                # G_h = wk_h^T @ Q_h^T  (K=16 contraction at base partition 0)
                for a in range(2):
                    heads = g_s if a == 0 else g_c
                    for h in range(H):
                        gp = pset.tile([128, 512], F32, name=f"gp{a}{h}",
                                       tag="gps")
                        nc.tensor.matmul(
                            gp[:],
                            lhsT=wk_t[a][:, 128 * h : 128 * (h + 1)],
                            rhs=qth[a][h][:],
                            start=True, stop=True,
                        )
                        nc.scalar.copy(out=heads[h][:], in_=gp[:])

                # V_aug packed: x @ [wv_packs for all 4 (attn, grp)] per k-tile
                for t in range(KT):
                    vp = pset.tile([128, 512], F32, name=f"vp{t}", tag="vps")
                    nc.tensor.matmul(
                        vp[:],
                        lhsT=xbT[:, 128 * t : 128 * (t + 1)],
                        rhs=wv_t[:],
                        start=True, stop=True,
                    )
                    nc.vector.tensor_copy(out=v_all[:, t, :], in_=vp[:])
                # ones columns for the softmax-denominator rows
                nc.sync.dma_start(
                    out=v_all[:].rearrange("p t (c x) -> p t c x", x=32)[:, :, :, 16],
                    in_=ones_v[:].rearrange("p (t c) -> p t c", c=16),
                )

            # ---------------- attention loops
            def attention(ai, g_heads, acc_tag, pa):
                """Process 4 sets of 2 heads; each set runs all 16 k-tiles
                with a double-buffered [128, 1024] score tile (2 banks) and a
                per-head PSUM accumulator bank; accumulators drain into the
                packed [128, 512] layout via 32-aligned DVE copies."""
                packed = [pp.tile([128, 512], F32, name=f"acc{ai}{g}",
                                  tag=f"{acc_tag}{g}") for g in range(2)]
                with tc.tile_pool(name=f"ebp{ai}", bufs=3) as ebp:
                    for st in range(4):
                        h0 = 2 * st
                        pv = [pa.tile([32, 512], F32, name=f"pv{ai}{st}{i}",
                                      tag=f"pv{i}") for i in range(2)]
                        for t in range(KT):
                            sc = pa.tile([128, 1024], F32, bufs=2,
                                         name=f"sc{ai}{st}{t}", tag="sc")
                            for i in range(2):
                                nc.tensor.matmul(
                                    sc[:, 512 * i : 512 * (i + 1)],
                                    lhsT=xbT[:, 128 * t : 128 * (t + 1)],
                                    rhs=g_heads[h0 + i][:],
                                    start=True, stop=True,
                                )
                            eb = ebp.tile([128, 1024], F32R, name="eb",
                                          tag="eb")
                            nc.scalar.activation(eb[:], sc[:], AF.Exp,
                                                 scale=0.25)
                            for i in range(2):
                                h = h0 + i
                                v0 = 256 * ai + 128 * (h // 4) + 32 * (h % 4)
                                nc.tensor.matmul(
                                    pv[i][:],
                                    lhsT=v_all[:, t, v0 : v0 + 32],
                                    rhs=eb[:, 512 * i : 512 * (i + 1)],
                                    start=(t == 0), stop=(t == KT - 1),
                                    skip_group_check=True,
                                )
                        # drain the two head accumulators into packed layout
                        for i in range(2):
                            h = h0 + i
                            nc.vector.tensor_copy(
                                out=packed[h // 4][32 * (h % 4) : 32 * (h % 4) + 32, :],
                                in_=pv[i][:],
                            )
                return packed

            def normalize_and_project(ai, accs, wp_t, pa):
                sa_n = []
                for grp in range(2):
                    sbc = pa.tile([128, 512], F32, name=f"sbc{ai}{grp}",
                                  tag=f"ps{grp}")
                    nc.tensor.matmul(sbc[:], lhsT=sel_t[:], rhs=accs[grp][:],
                                     start=True, stop=True)
                    rb = pp.tile([128, 512], F32, name=f"rb{ai}{grp}",
                                 tag=f"rb{grp}")
                    nc.vector.reciprocal(out=rb[:], in_=sbc[:])
                    sn = pp.tile([128, 512], F32R, name=f"sn{ai}{grp}",
                                 tag=f"sn{grp}")
                    nc.vector.tensor_mul(sn[:], accs[grp][:], rb[:])
                    sa_n.append(sn)
                rp = pa.tile([128, 512], F32, name=f"rp{ai}", tag="ps0")
                for grp in range(2):
                    nc.tensor.matmul(rp[:], lhsT=wp_t[grp][:],
                                     rhs=sa_n[grp][:],
                                     start=(grp == 0), stop=(grp == 1))
                return rp

            with tc.tile_pool(name="pattn", bufs=1, space="PSUM") as pa:
                acc_s = attention(0, g_s, "acs", pa)
                rp1 = normalize_and_project(0, acc_s, w1p_t, pa)
                r1T = sbuf("r1T", (128, 512))
                nc.vector.tensor_add(r1T[:], rp1[:], xqT[:])
                nc.vector.tensor_scalar_add(r1T[:], r1T[:], b1_t[:])

                acc_c = attention(1, g_c, "acc", pa)
                rp2 = normalize_and_project(1, acc_c, w2p_t, pa)
                r2T = sbuf("r2T", (128, 512))
                nc.vector.tensor_add(r2T[:], rp2[:], r1T[:])
                nc.vector.tensor_scalar_add(r2T[:], r2T[:], b2_t[:])

            # ---------------- layernorm (x - m) / var, var unbiased
            with tc.tile_pool(name="ptail", bufs=1, space="PSUM") as pt:
                sq = sbuf("sq", (128, 512))
                nc.vector.tensor_mul(sq[:], r2T[:], r2T[:])
                mp = pt.tile([1, 512], F32, name="mp", tag="st0")
                nc.tensor.matmul(mp[:], lhsT=onec_t[:], rhs=r2T[:],
                                 start=True, stop=True)
                sp = pt.tile([1, 512], F32, name="sp", tag="st1")
                nc.tensor.matmul(sp[:], lhsT=onec_t[:], rhs=sq[:],
                                 start=True, stop=True)
                msb = sbuf("msb", (1, 512))
                nc.vector.tensor_copy(out=msb[:], in_=mp[:])
                ssb = sbuf("ssb", (1, 512))
                nc.vector.tensor_copy(out=ssb[:], in_=sp[:])
                t0 = sbuf("t0", (1, 512))
                nc.vector.tensor_mul(t0[:], msb[:], msb[:])
                nc.vector.tensor_scalar_mul(t0[:], t0[:], 1.0 / 128)
                nc.vector.tensor_sub(t0[:], ssb[:], t0[:])  # sum((x-m)^2)
                asb = sbuf("asb", (1, 512))
                nc.vector.reciprocal(out=asb[:], in_=t0[:])
                nc.vector.tensor_scalar_mul(asb[:], asb[:], 127.0)  # a = 1/var
                bsb = sbuf("bsb", (1, 512))
                nc.vector.tensor_mul(bsb[:], msb[:], asb[:])
                nc.vector.tensor_scalar_mul(bsb[:], bsb[:], -1.0 / 128)  # -m/var
                abc = pt.tile([128, 512], F32, name="abc", tag="bc0")
                nc.tensor.matmul(abc[:], lhsT=oner_t[:], rhs=asb[:],
                                 start=True, stop=True)
                bbc = pt.tile([128, 512], F32, name="bbc", tag="bc1")
                nc.tensor.matmul(bbc[:], lhsT=oner_t[:], rhs=bsb[:],
                                 start=True, stop=True)
                lnT = sbuf("lnT", (128, 512), F32R)
                nc.vector.tensor_mul(lnT[:], r2T[:], abc[:])
                nc.vector.tensor_add(lnT[:], lnT[:], bbc[:])

                # ---------------- FFN
                h_sb = []
                for j in range(4):
                    hp = pt.tile([128, 512], F32, name=f"hp{j}", tag=f"hp{j % 2}")
                    nc.tensor.matmul(hp[:],
                                     lhsT=w3_t[:, 128 * j : 128 * (j + 1)],
                                     rhs=lnT[:], start=True, stop=True)
                    hs = sbuf(f"hs{j}", (128, 512), F32R)
                    nc.vector.tensor_scalar(
                        out=hs[:], in0=hp[:], scalar1=b3_t[:, j : j + 1],
                        scalar2=0.0, op0=OP.add, op1=OP.max,
                    )
                    h_sb.append(hs)
                op_ = pt.tile([128, 512], F32, name="op", tag="bc0")
                for j in range(4):
                    nc.tensor.matmul(op_[:],
                                     lhsT=w4_t[:, 128 * j : 128 * (j + 1)],
                                     rhs=h_sb[j][:],
                                     start=(j == 0), stop=(j == 3),
                                     skip_group_check=True)
                oT = sbuf("oT", (128, 512))
                nc.vector.tensor_add(oT[:], op_[:], r2T[:])
                nc.vector.tensor_scalar_add(oT[:], oT[:], b4_t[:])

                # ------------- transpose back to natural rows and store
                on = sbuf("on", (128, 512))
                for j in range(4):
                    tp = pt.tile([128, 128], F32, name=f"tp{j}", tag=f"st{j % 2}")
                    nc.tensor.transpose(tp[:], oT[:, 128 * j : 128 * (j + 1)],
                                        id_t[:])
                    nc.vector.tensor_copy(out=on[:, 128 * j : 128 * (j + 1)],
                                          in_=tp[:])
                nc.sync.dma_start(out=y[:], in_=on[:])

    return nc


_CACHED = {}


def _get_nc():
    if "nc" not in _CACHED:
        _CACHED["nc"] = _split_multiwaits(build_nc())
    return _CACHED["nc"]


def _host_inputs(x_tgt, enc_out, self_wq, self_wk, self_wv, cross_wq, cross_wk,
                 cross_wv, w1, b1, w2, b2, w3, b3, w4, b4):
    shared = {
        "wqh0": _perm_head_major(self_wq), "wqh1": _perm_head_major(cross_wq),
        "wkh0": _wk_head_T(self_wk), "wkh1": _wk_head_T(cross_wk),
        "wv_st": np.concatenate(
            [_pack32_cols(self_wv, 0), _pack32_cols(self_wv, 1),
             _pack32_cols(cross_wv, 0), _pack32_cols(cross_wv, 1)], axis=1
        ),
        "w1p0": _pack_w12(w1, 0), "w1p1": _pack_w12(w1, 1),
        "w2p0": _pack_w12(w2, 0), "w2p1": _pack_w12(w2, 1),
        "w3": w3,
        "w4r": np.ascontiguousarray(
            w4.reshape(4, 128, 128).transpose(1, 0, 2).reshape(128, 512)
        ),
        "ones_v": np.ones((128, 256), np.float32),
        "selt": _sel_matrix(),
        "ident": np.eye(128, dtype=np.float32),
        "ones_col": np.ones((128, 1), np.float32),
        "ones_row": np.ones((1, 128), np.float32),
        "b1t": b1.reshape(128, 1),
        "b2t": b2.reshape(128, 1),
        "b3t": np.ascontiguousarray(b3.reshape(4, 128).T),
        "b4t": b4.reshape(128, 1),
    }
    shared = {k: np.ascontiguousarray(v, dtype=np.float32)
              for k, v in shared.items()}
    in_maps = []
    for c in range(NCORES):
        b, qb = divmod(c, 4)
        q0 = qb * QC
        im = dict(shared)
        im["xb"] = _shuf(x_tgt[b])
        im["xq"] = _shuf(x_tgt[b, q0 : q0 + QC])
        im["eo"] = _shuf(enc_out[b, q0 : q0 + QC])
        in_maps.append(im)
    return in_maps


def run_on_device(in_maps, **kw):
    nc = _get_nc()
    return run_bass_kernel_spmd(nc, in_maps, list(range(NCORES)), **kw)


def kernel(x_tgt, enc_out, self_wq, self_wk, self_wv, cross_wq, cross_wk,
           cross_wv, w1, b1, w2, b2, w3, b3, w4, b4, mask_src=None,
           mask_tgt=None, **_unused):
    args = [x_tgt, enc_out, self_wq, self_wk, self_wv, cross_wq, cross_wk,
            cross_wv, w1, b1, w2, b2, w3, b3, w4, b4]
    args = [np.asarray(a, dtype=np.float32) for a in args]
    in_maps = _host_inputs(*args)
    res = run_on_device(in_maps)
    out = np.empty((B, S, D), np.float32)
    for c in range(NCORES):
        b, qb = divmod(c, 4)
        out[b, qb * QC : (qb + 1) * QC] = _unshuf(res.results[c]["y"])
    return out



# revision 16
# speedup vs baseline: 1.0507x; 1.0507x over previous
"""Trainium2 Bass kernel for a (quirky) transformer decoder layer.

Problem shapes: B=2, S=2048, D=128, H=8 heads, head_dim=16.
  sa  = attn(q=x_tgt, kv=x_tgt);  r1 = sa @ w1 + b1 + x_tgt
  ca  = attn(q=enc_out, kv=x_tgt); r2 = ca @ w2 + b2 + r1
  ln  = (r2 - mean) / var   (var unbiased, divide by var not std)
  out = relu(ln @ w3 + b3) @ w4 + b4 + r2
(mask_src / mask_tgt are unused by the reference.)

Sharding: 8 cores, query-row sharding (zero communication). Core c handles
batch c//4, query rows [(c%4)*512 : (c%4+1)*512].

Attention core runs in fp8 (e4m3) with MatmulPerfMode.DoubleRow (0.5
cycles/column on the PE, 2x the fp32r rate):
 - scores: wq/wk are folded on host into A_h = 16*wk_h@wq_h^T, so
   scores^T = x_kv @ (A_h @ x_q^T) contracts over the 128 input channels,
   split [64, 2, .] for DoubleRow. A 65th channel row (x side = 8.0, G side
   = per-query hi/lo fp8 shift pair, host-computed from a rank-1 estimate of
   the per-query score max) subtracts an approximate softmax max so
   exp values fit fp8's +-240 range. The shift cancels exactly in the
   softmax ratio, so only fp8 representability (not accuracy) depends on it.
 - exp runs on ScalarE (the only engine with exp), fp8 output, one
   [128, 1024] activation per (head, key-tile-pair). Scalar is deliberately
   stripped of all other work (copies live on DVE) since exp is the
   engine-time floor of the kernel.
 - PV: per head, 8 DoubleRow matmuls contract key-tile pairs; a ones column
   inside the packed V (col 16 of each 32-col head group) accumulates the
   softmax denominator. 4 heads accumulate into one PSUM bank at 32-partition
   offsets (tile_position column groups).
All activations stay transposed [d, token] on device; the host uploads
x^T directly and un-transposes the output, so the kernel has zero PE
transposes.
"""

import numpy as np
import ml_dtypes

import concourse.bass as bass
import concourse.tile as tile
from concourse import mybir
from concourse.bass_utils import run_bass_kernel_spmd

B, S, D, H, HD = 2, 2048, 128, 8, 16
QC = 512  # query rows per core
NCORES = 8
KT = 16  # number of 128-row key tiles
F32 = mybir.dt.float32
F32R = mybir.dt.float32r
FP8 = mybir.dt.float8e4
AF = mybir.ActivationFunctionType
OP = mybir.AluOpType
DR = mybir.MatmulPerfMode.DoubleRow
E4NP = ml_dtypes.float8_e4m3
KAPPA = 1.5  # headroom above the estimated max: top exp value ~ e^(est_err+KAPPA)


# ---------------------------------------------------------------- host packing
def _pack32_cols(w, grp):
    """[D, 128]: col 32g+j (j<16) = w[:, j*H + (4*grp+g)], else 0 (col 16 of
    each 32-group is later filled with 1.0 for the softmax denominator)."""
    out = np.zeros((D, 128), np.float32)
    for g in range(4):
        h = 4 * grp + g
        for j in range(HD):
            out[:, 32 * g + j] = w[:, j * H + h]
    return out


def _pack_w12(w, grp):
    """lhsT for the merge projection: row 32c+j = w[j*H + (4*grp+c), :]."""
    out = np.zeros((D, D), np.float32)
    for c in range(4):
        h = 4 * grp + c
        for j in range(HD):
            out[32 * c + j, :] = w[j * H + h, :]
    return out


def _sel_matrix():
    sel = np.zeros((128, 128), np.float32)
    for m in range(128):
        sel[32 * (m // 32) + 16, m] = 1.0
    return sel


def _split_multiwaits(nc):
    """Post-pass for walrus builds that accept only ONE sync-wait per
    instruction: split every instruction carrying N>1 waits into (N-1)
    single-wait NOPs on the same engine placed immediately before it."""
    uid = 0
    for f in nc.m.functions:
        for bb in f.blocks:
            il = bb.instructions
            if not any(
                i.sync_info is not None
                and i.sync_info.on_wait
                and len(i.sync_info.on_wait) > 1
                for i in il
            ):
                continue
            out = []
            for inst in il:
                si = inst.sync_info
                if si is not None and si.on_wait and len(si.on_wait) > 1:
                    waits = list(si.on_wait)
                    for w in waits[:-1]:
                        uid += 1
                        nop = mybir.InstNoOp(
                            name=f"WSPLIT-{uid}",
                            engine=inst.engine,
                            ins=[],
                            outs=[],
                            sync_info=mybir.SyncInfo(on_wait=[w], on_update=[]),
                        )
                        out.append(nop)
                    inst.sync_info = mybir.SyncInfo(
                        on_wait=[waits[-1]], on_update=list(si.on_update)
                    )
                out.append(inst)
            bb.instructions = out
    return nc


# ---------------------------------------------------------------- device build
def build_nc(debug=False):
    nc = bass.Bass()

    def din(name, shape, dt=F32R):
        return nc.dram_tensor(name, list(shape), dt, kind="ExternalInput")

    xbT = din("xbT", (128, 2048))  # batch x_tgt transposed [channel, key]
    xqT = din("xqT", (128, 512))  # query slice of x_tgt, transposed
    eoT = din("eoT", (128, 512))  # query slice of enc_out, transposed
    xb8 = din("xb8", (65, KT, 2, 128), FP8)  # fp8 keys + shift channel row 64
    at = din("at", (128, 16 * 128))  # A_h^T stacked per (attn*8+h)
    c8 = din("c8", (1, 16, 2, 512), FP8)  # per-query shift rows (hi/lo)
    wv_st = din("wv_st", (D, 512))  # [v_selfA | v_selfB | v_crossA | v_crossB]
    ones_v = din("ones_v", (128, 256), FP8)  # denominator columns for V
    w1p = [din(f"w1p{g}", (D, D)) for g in range(2)]
    w2p = [din(f"w2p{g}", (D, D)) for g in range(2)]
    w3 = din("w3", (D, 512))
    w4r = din("w4r", (128, 512))  # col block j = w4[128j:128j+128, :]
    selt = din("selt", (128, 128))  # SEL[p, m] = (p == 32*(m//32)+16)
    ones_col = din("ones_col", (128, 1))
    ones_row = din("ones_row", (1, 128))
    b1t = din("b1t", (128, 1), F32)
    b2t = din("b2t", (128, 1), F32)
    b3t = din("b3t", (128, 4), F32)
    b4t = din("b4t", (128, 1), F32)
    y = nc.dram_tensor("y", [128, 512], F32, kind="ExternalOutput")
    if debug:
        dbg = {
            "g8": nc.dram_tensor("dbg_g8", [65, 16, 2, 512], FP8,
                                 kind="ExternalOutput"),
            "v8": nc.dram_tensor("dbg_v8", [128, 2, 512], FP8,
                                 kind="ExternalOutput"),
            "eb": nc.dram_tensor("dbg_eb", [128, 2, 512], FP8,
                                 kind="ExternalOutput"),
            "sc": nc.dram_tensor("dbg_sc", [128, 1024], F32,
                                 kind="ExternalOutput"),
            "acc": nc.dram_tensor("dbg_acc", [128, 512], F32,
                                  kind="ExternalOutput"),
        }

    with tile.TileContext(nc) as tc:
        with tc.tile_pool(name="persist", bufs=1) as pp:

            def sbuf(name, shape, dt=F32):
                return pp.tile(list(shape), dt, name=name, tag=name)

            def load(name, dram, shape, dt=F32R):
                t = sbuf(name, shape, dt)
                nc.sync.dma_start(out=t[:], in_=dram[:])
                return t

            # ---- loads ordered by first use
            xqT_t = load("xqT", xqT, (128, 512))
            at_t = load("at", at, (128, 16 * 128))
            eoT_t = load("eoT", eoT, (128, 512))
            xb8_t = load("xb8", xb8, (65, KT, 2, 128), FP8)
            xbT_t = load("xbT", xbT, (128, 2048))
            wv_t = load("wv", wv_st, (D, 512))
            w1p_t = [load(f"w1p{g}", w1p[g], (D, D)) for g in range(2)]
            w2p_t = [load(f"w2p{g}", w2p[g], (D, D)) for g in range(2)]
            w3_t = load("w3", w3, (D, 512))
            w4_t = load("w4", w4r, (128, 512))
            sel_t = load("sel", selt, (128, 128))
            onec_t = load("onec", ones_col, (128, 1))
            oner_t = load("oner", ones_row, (1, 128))
            b1_t = load("b1", b1t, (128, 1), F32)
            b2_t = load("b2", b2t, (128, 1), F32)
            b3_t = load("b3", b3t, (128, 4), F32)
            b4_t = load("b4", b4t, (128, 1), F32)

            # fp8 G (scores rhs): rows 0-63 from PSUM, row 64 = shift pair
            g8 = sbuf("g8", (65, 16, 2, 512), FP8)
            nc.sync.dma_start(out=g8[64:65, :, :, :], in_=c8[:])
            # fp8 packed V, one tile per key-tile pair
            v8 = [sbuf(f"v8p{p}", (128, 2, 512), FP8) for p in range(8)]

            # ---------------- setup: G projections + V projections
            with tc.tile_pool(name="pset", bufs=2, space="PSUM") as pset:
                for ah in range(16):  # attn*8 + head
                    xsrc = xqT_t if ah < 8 else eoT_t
                    gp = pset.tile([64, 1024], F32, name=f"gp{ah}", tag="gp")
                    for i in range(2):
                        nc.tensor.matmul(
                            gp[:, 512 * i : 512 * (i + 1)],
                            lhsT=at_t[:, 128 * ah + 64 * i : 128 * ah + 64 * (i + 1)],
                            rhs=xsrc[:],
                            start=True,
                            stop=True,
                        )
                    nc.vector.tensor_copy(out=g8[0:64, ah, :, :], in_=gp[:])
                for t in range(KT):
                    vp = pset.tile([128, 512], F32, name=f"vp{t}", tag="vp")
                    nc.tensor.matmul(
                        vp[:],
                        lhsT=xbT_t[:, 128 * t : 128 * (t + 1)],
                        rhs=wv_t[:],
                        start=True,
                        stop=True,
                    )
                    nc.vector.tensor_copy(out=v8[t // 2][:, t % 2, :], in_=vp[:])
                    if t % 2 == 1:
                        # denominator ones columns, after the V copies so they
                        # are not overwritten by the zero-padded projection
                        p = t // 2
                        nc.sync.dma_start(
                            out=v8[p][:].rearrange(
                                "p b (c x) -> p b c x", x=32
                            )[:, :, :, 16],
                            in_=ones_v[:, 32 * p : 32 * (p + 1)].rearrange(
                                "p (b c) -> p b c", b=2
                            ),
                        )

            # ---------------- attention
            acc_sb = [[sbuf(f"acc{a}{g}", (128, 512), F32R) for g in range(2)]
                      for a in range(2)]

            def attention(a, pa, ebp):
                """per head: 8 key-tile-pair steps of [2 score DoubleRow mms
                -> exp(fp8 out) -> 1 pv DoubleRow]; head pairs accumulate in
                one PSUM bank at partition 0/64 (DoubleRow dst granularity),
                drained as two 32-row strips into the packed accumulator."""
                for h in range(H):
                    ah = 8 * a + h
                    g, hh = h // 4, h % 4
                    c0 = 256 * a + 128 * g + 32 * hh
                    pv = pa.tile([32, 512], F32, name=f"pv{a}{h}",
                                 tag=f"pv{h % 2}")
                    for p in range(8):
                        sc = pa.tile([128, 1024], F32, bufs=2,
                                     name=f"sc{ah}{p}", tag="sc")
                        for i in range(2):
                            nc.tensor.matmul(
                                sc[:, 512 * i : 512 * (i + 1)],
                                lhsT=xb8_t[:, 2 * p + i, :, :],
                                rhs=g8[:, ah, :, :],
                                start=True,
                                stop=True,
                                perf_mode=DR,
                            )
                        eb = ebp.tile([128, 2, 512], FP8, name="eb", tag="eb")
                        with nc.allow_low_precision(reason="fp8 softmax"):
                            nc.scalar.activation(
                                eb[:].rearrange("p b x -> p (b x)"),
                                sc[:],
                                AF.Exp,
                                scale=0.015625,
                            )
                        if debug and a == 0 and h == 0 and p == 0:
                            scs = pp.tile([128, 1024], F32, name="dbgscs",
                                          tag="dbgscs")
                            nc.vector.tensor_copy(out=scs[:], in_=sc[:])
                            nc.sync.dma_start(out=dbg["sc"][:], in_=scs[:])
                            nc.sync.dma_start(out=dbg["eb"][:], in_=eb[:])
                        nc.tensor.matmul(
                            pv[:],
                            lhsT=v8[p][:, :, c0 : c0 + 32],
                            rhs=eb[:],
                            start=(p == 0),
                            stop=(p == 7),
                            perf_mode=DR,
                            skip_group_check=True,
                        )
                    nc.vector.tensor_copy(
                        out=acc_sb[a][g][32 * hh : 32 * hh + 32, :],
                        in_=pv[:],
                    )

            def normalize_and_project(a, wp_t, pa):
                sa_n = []
                for g in range(2):
                    sbc = pa.tile([128, 512], F32, name=f"sbc{a}{g}",
                                  tag=f"nm{g}")
                    nc.tensor.matmul(sbc[:], lhsT=sel_t[:], rhs=acc_sb[a][g][:],
                                     start=True, stop=True)
                    rb = pp.tile([128, 512], F32, name=f"rb{a}{g}",
                                 tag=f"rb{g}")
                    nc.vector.reciprocal(out=rb[:], in_=sbc[:])
                    sn = pp.tile([128, 512], F32R, name=f"sn{a}{g}",
                                 tag=f"sn{g}")
                    nc.vector.tensor_mul(sn[:], acc_sb[a][g][:], rb[:])
                    sa_n.append(sn)
                rp = pa.tile([128, 512], F32, name=f"rp{a}", tag="nm0")
                for g in range(2):
                    nc.tensor.matmul(rp[:], lhsT=wp_t[g][:], rhs=sa_n[g][:],
                                     start=(g == 0), stop=(g == 1))
                return rp

            with tc.tile_pool(name="pattn", bufs=1, space="PSUM") as pa, \
                 tc.tile_pool(name="ebp", bufs=3) as ebp:
                attention(0, pa, ebp)
                if debug:
                    nc.sync.dma_start(out=dbg["g8"][:], in_=g8[:])
                    nc.sync.dma_start(out=dbg["v8"][:], in_=v8[0][:])
                    nc.sync.dma_start(out=dbg["acc"][:], in_=acc_sb[0][0][:].bitcast(F32))
                rp1 = normalize_and_project(0, w1p_t, pa)
                r1T = sbuf("r1T", (128, 512))
                nc.vector.tensor_add(r1T[:], rp1[:], xqT_t[:])
                nc.vector.tensor_scalar_add(r1T[:], r1T[:], b1_t[:])

                attention(1, pa, ebp)
                rp2 = normalize_and_project(1, w2p_t, pa)
                r2T = sbuf("r2T", (128, 512), F32R)
                nc.vector.tensor_add(r2T[:], rp2[:], r1T[:])
                nc.vector.tensor_scalar_add(r2T[:], r2T[:], b2_t[:])

            # ---------------- layernorm (x - m) / var, var unbiased
            with tc.tile_pool(name="ptail", bufs=1, space="PSUM") as pt:
                sq = sbuf("sq", (128, 512), F32R)
                nc.vector.tensor_mul(sq[:], r2T[:], r2T[:])
                mp = pt.tile([1, 512], F32, name="mp", tag="st0")
                nc.tensor.matmul(mp[:], lhsT=onec_t[:], rhs=r2T[:],
                                 start=True, stop=True)
                sp = pt.tile([1, 512], F32, name="sp", tag="st1")
                nc.tensor.matmul(sp[:], lhsT=onec_t[:], rhs=sq[:],
                                 start=True, stop=True)
                msb = sbuf("msb", (1, 512))
                nc.vector.tensor_copy(out=msb[:], in_=mp[:])
                ssb = sbuf("ssb", (1, 512))
                nc.vector.tensor_copy(out=ssb[:], in_=sp[:])
                t0 = sbuf("t0", (1, 512))
                nc.vector.tensor_mul(t0[:], msb[:], msb[:])
                nc.vector.tensor_scalar_mul(t0[:], t0[:], 1.0 / 128)
                nc.vector.tensor_sub(t0[:], ssb[:], t0[:])  # sum((x-m)^2)
                asb = sbuf("asb", (1, 512), F32R)
                with nc.allow_low_precision(reason="f32r is full fp32"):
                    nc.vector.reciprocal(out=asb[:], in_=t0[:])
                nc.vector.tensor_scalar_mul(asb[:], asb[:], 127.0)  # 1/var
                bsb = sbuf("bsb", (1, 512), F32R)
                nc.vector.tensor_mul(bsb[:], msb[:], asb[:])
                nc.vector.tensor_scalar_mul(bsb[:], bsb[:], -1.0 / 128)
                abc = pt.tile([128, 512], F32, name="abc", tag="bc0")
                nc.tensor.matmul(abc[:], lhsT=oner_t[:], rhs=asb[:],
                                 start=True, stop=True)
                bbc = pt.tile([128, 512], F32, name="bbc", tag="bc1")
                nc.tensor.matmul(bbc[:], lhsT=oner_t[:], rhs=bsb[:],
                                 start=True, stop=True)
                lnT = sbuf("lnT", (128, 512), F32R)
                nc.vector.tensor_mul(lnT[:], r2T[:], abc[:])
                nc.vector.tensor_add(lnT[:], lnT[:], bbc[:])

                # ---------------- FFN
                h_sb = []
                for j in range(4):
                    hp = pt.tile([128, 512], F32, name=f"hp{j}", tag=f"hp{j % 2}")
                    nc.tensor.matmul(hp[:],
                                     lhsT=w3_t[:, 128 * j : 128 * (j + 1)],
                                     rhs=lnT[:], start=True, stop=True)
                    hs = sbuf(f"hs{j}", (128, 512), F32R)
                    nc.vector.tensor_scalar(
                        out=hs[:], in0=hp[:], scalar1=b3_t[:, j : j + 1],
                        scalar2=0.0, op0=OP.add, op1=OP.max,
                    )
                    h_sb.append(hs)
                op_ = pt.tile([128, 512], F32, name="op", tag="bc0")
                for j in range(4):
                    nc.tensor.matmul(op_[:],
                                     lhsT=w4_t[:, 128 * j : 128 * (j + 1)],
                                     rhs=h_sb[j][:],
                                     start=(j == 0), stop=(j == 3),
                                     skip_group_check=True)
                oT = sbuf("oT", (128, 512))
                nc.vector.tensor_add(oT[:], op_[:], r2T[:])
                nc.vector.tensor_scalar_add(oT[:], oT[:], b4_t[:])
                nc.sync.dma_start(out=y[:], in_=oT[:])

    return nc


_CACHED = {}


def _get_nc():
    if "nc" not in _CACHED:
        _CACHED["nc"] = _split_multiwaits(build_nc())
    return _CACHED["nc"]


def _host_inputs(x_tgt, enc_out, self_wq, self_wk, self_wv, cross_wq, cross_wk,
                 cross_wv, w1, b1, w2, b2, w3, b3, w4, b4):
    # folded score matrices A_h (x16 so fp8 G has good SNR; exp scale 1/64)
    at = np.zeros((2, H, D, D), np.float32)
    lam = np.zeros((2, H), np.float32)
    for a, (wq, wk) in enumerate(((self_wq, self_wk), (cross_wq, cross_wk))):
        for h in range(H):
            A = 16.0 * (wk[:, h::H] @ wq[:, h::H].T)
            at[a, h] = A.T
            lam[a, h] = A.mean() / 64.0
    at_flat = np.ascontiguousarray(
        at.reshape(16, D, D).transpose(1, 0, 2).reshape(D, 16 * D)
    )

    shared = {
        "at": at_flat,
        "wv_st": np.concatenate(
            [_pack32_cols(self_wv, 0), _pack32_cols(self_wv, 1),
             _pack32_cols(cross_wv, 0), _pack32_cols(cross_wv, 1)], axis=1
        ),
        "ones_v": np.ones((128, 256), E4NP),
        "w1p0": _pack_w12(w1, 0), "w1p1": _pack_w12(w1, 1),
        "w2p0": _pack_w12(w2, 0), "w2p1": _pack_w12(w2, 1),
        "w3": w3,
        "w4r": np.ascontiguousarray(
            w4.reshape(4, 128, 128).transpose(1, 0, 2).reshape(128, 512)
        ),
        "selt": _sel_matrix(),
        "ones_col": np.ones((128, 1), np.float32),
        "ones_row": np.ones((1, 128), np.float32),
        "b1t": b1.reshape(128, 1),
        "b2t": b2.reshape(128, 1),
        "b3t": np.ascontiguousarray(b3.reshape(4, 128).T),
        "b4t": b4.reshape(128, 1),
    }
    shared = {k: (v if v.dtype == E4NP else
                  np.ascontiguousarray(v, dtype=np.float32))
              for k, v in shared.items()}

    # per-batch fp8 keys (+ shift channel row 64 = 8.0)
    xb8_b = []
    u_ext = []
    for b in range(B):
        xb = np.zeros((65, KT, 2, 128), E4NP)
        xs = x_tgt[b].astype(E4NP)  # (2048, 128)
        for t in range(KT):
            blk = xs[128 * t : 128 * (t + 1), :]  # (128k, 128c)
            xb[0:64, t, 0, :] = blk[:, 0:64].T
            xb[0:64, t, 1, :] = blk[:, 64:128].T
        xb[64, :, :, :] = np.float32(8.0)
        xb8_b.append(xb)
        u_k = x_tgt[b].sum(-1)
        u_ext.append((u_k.min(), u_k.max()))

    in_maps = []
    for c in range(NCORES):
        b, qb = divmod(c, 4)
        q0 = qb * QC
        im = dict(shared)
        im["xbT"] = np.ascontiguousarray(x_tgt[b].T)
        im["xqT"] = np.ascontiguousarray(x_tgt[b, q0 : q0 + QC].T)
        im["eoT"] = np.ascontiguousarray(enc_out[b, q0 : q0 + QC].T)
        im["xb8"] = xb8_b[b]
        # shift rows: est = rank-1 estimate of per-query max score
        umin, umax = u_ext[b]
        c8 = np.zeros((1, 16, 2, 512), E4NP)
        for a in range(2):
            u_q = (x_tgt if a == 0 else enc_out)[b, q0 : q0 + QC].sum(-1)
            for h in range(H):
                l = lam[a, h]
                est = l * np.where(l * u_q > 0, umax * u_q, umin * u_q)
                tot = 8.0 * (KAPPA - est)
                g0 = (tot * 0.5).astype(E4NP)
                g1 = (tot - g0.astype(np.float32)).astype(E4NP)
                c8[0, 8 * a + h, 0, :] = g0
                c8[0, 8 * a + h, 1, :] = g1
        im["c8"] = c8
        in_maps.append(im)
    return in_maps


def run_on_device(in_maps, **kw):
    nc = _get_nc()
    return run_bass_kernel_spmd(nc, in_maps, list(range(NCORES)), **kw)


def kernel(x_tgt, enc_out, self_wq, self_wk, self_wv, cross_wq, cross_wk,
           cross_wv, w1, b1, w2, b2, w3, b3, w4, b4, mask_src=None,
           mask_tgt=None, **_unused):
    args = [x_tgt, enc_out, self_wq, self_wk, self_wv, cross_wq, cross_wk,
            cross_wv, w1, b1, w2, b2, w3, b3, w4, b4]
    args = [np.asarray(a, dtype=np.float32) for a in args]
    in_maps = _host_inputs(*args)
    res = run_on_device(in_maps)
    out = np.empty((B, S, D), np.float32)
    for c in range(NCORES):
        b, qb = divmod(c, 4)
        out[b, qb * QC : (qb + 1) * QC] = res.results[c]["y"].T
    return out


# revision 19
# speedup vs baseline: 1.1527x; 1.0971x over previous
"""Trainium2 Bass kernel for a (quirky) transformer decoder layer.

Problem shapes: B=2, S=2048, D=128, H=8 heads, head_dim=16.
  sa  = attn(q=x_tgt, kv=x_tgt);  r1 = sa @ w1 + b1 + x_tgt
  ca  = attn(q=enc_out, kv=x_tgt); r2 = ca @ w2 + b2 + r1
  ln  = (r2 - mean) / var   (var unbiased, divide by var not std)
  out = relu(ln @ w3 + b3) @ w4 + b4 + r2
(mask_src / mask_tgt are unused by the reference.)

Sharding: 8 cores, query-row sharding (zero communication). Core c handles
batch c//4, query rows [(c%4)*512 : (c%4+1)*512].

Measured HW facts this kernel is shaped around (micro-benched):
 - A [K=128, M=128, N=512] fp32r matmul sustains ~320ns; the same columns
   in fp8/bf16/DoubleRow run no faster (PE is power/issue-limited, so
   exotic dtypes buy nothing; fp32r keeps full accuracy).
 - Small-K matmuls are SLOWER per column (K=16 -> ~558ns), so scores keep
   the full-128-channel contraction via the host-folded A_h = wk_h@wq_h^T
   (scores^T = x_kv @ (A_h @ x_q^T)), which also removes all Q/K
   projections and their PSUM->SBUF copies.
 - exp on ScalarE costs ~1.08us per [128, 1024] tile regardless of dtype;
   Scalar runs ONLY exp (160 activations), everything else lives on DVE.
All activations stay transposed [d, token] on device; the host uploads
x^T directly and un-transposes the output, so the kernel has zero PE
transposes. Softmax skips max-subtraction (exp in fp32: scores reach ~33,
e^33 ~ 2e14 is finite and the denominator ratio is exact); the denominator
rides the packed V as a ones column (col 16 of each 32-col head group) and
is broadcast via a selector matmul + DVE reciprocal.
"""

import numpy as np

import concourse.bass as bass
import concourse.tile as tile
from concourse import mybir
from concourse.bass_utils import run_bass_kernel_spmd

B, S, D, H, HD = 2, 2048, 128, 8, 16
QC = 512  # query rows per core
NCORES = 8
KT = 16  # number of 128-row key tiles
F32 = mybir.dt.float32
F32R = mybir.dt.float32r
AF = mybir.ActivationFunctionType
OP = mybir.AluOpType


# ---------------------------------------------------------------- host packing
def _pack32_cols(w, grp):
    """[D, 128]: col 32g+j (j<16) = w[:, j*H + (4*grp+g)], else 0 (col 16 of
    each 32-group is later filled with 1.0 for the softmax denominator)."""
    out = np.zeros((D, 128), np.float32)
    for g in range(4):
        h = 4 * grp + g
        for j in range(HD):
            out[:, 32 * g + j] = w[:, j * H + h]
    return out


def _pack_w12(w, grp):
    """lhsT for the merge projection: row 32c+j = w[j*H + (4*grp+c), :]."""
    out = np.zeros((D, D), np.float32)
    for c in range(4):
        h = 4 * grp + c
        for j in range(HD):
            out[32 * c + j, :] = w[j * H + h, :]
    return out


def _sel_matrix():
    sel = np.zeros((128, 128), np.float32)
    for m in range(128):
        sel[32 * (m // 32) + 16, m] = 1.0
    return sel


def _split_multiwaits(nc):
    """Post-pass for walrus builds that accept only ONE sync-wait per
    instruction: split every instruction carrying N>1 waits into (N-1)
    single-wait NOPs on the same engine placed immediately before it."""
    uid = 0
    for f in nc.m.functions:
        for bb in f.blocks:
            il = bb.instructions
            if not any(
                i.sync_info is not None
                and i.sync_info.on_wait
                and len(i.sync_info.on_wait) > 1
                for i in il
            ):
                continue
            out = []
            for inst in il:
                si = inst.sync_info
                if si is not None and si.on_wait and len(si.on_wait) > 1:
                    waits = list(si.on_wait)
                    for w in waits[:-1]:
                        uid += 1
                        nop = mybir.InstNoOp(
                            name=f"WSPLIT-{uid}",
                            engine=inst.engine,
                            ins=[],
                            outs=[],
                            sync_info=mybir.SyncInfo(on_wait=[w], on_update=[]),
                        )
                        out.append(nop)
                    inst.sync_info = mybir.SyncInfo(
                        on_wait=[waits[-1]], on_update=list(si.on_update)
                    )
                out.append(inst)
            bb.instructions = out
    return nc


# ---------------------------------------------------------------- device build
def build_nc():
    nc = bass.Bass()

    def din(name, shape, dt=F32R):
        return nc.dram_tensor(name, list(shape), dt, kind="ExternalInput")

    xbT = din("xbT", (128, 2048))  # batch x_tgt transposed [channel, key]
    xqT = din("xqT", (128, 512))  # query slice of x_tgt, transposed
    eoT = din("eoT", (128, 512))  # query slice of enc_out, transposed
    at = din("at", (128, 16 * 128))  # A_h^T stacked per (attn*8+h)
    wv_st = din("wv_st", (D, 512))  # [v_selfA | v_selfB | v_crossA | v_crossB]
    ones_v = din("ones_v", (128, 256))  # denominator columns for V
    w1p = [din(f"w1p{g}", (D, D)) for g in range(2)]
    w2p = [din(f"w2p{g}", (D, D)) for g in range(2)]
    w3 = din("w3", (D, 512))
    w4r = din("w4r", (128, 512))  # col block j = w4[128j:128j+128, :]
    selt = din("selt", (128, 128))  # SEL[p, m] = (p == 32*(m//32)+16)
    ones_col = din("ones_col", (128, 1))
    ones_row = din("ones_row", (1, 128))
    b1t = din("b1t", (128, 1), F32)
    b2t = din("b2t", (128, 1), F32)
    b3t = din("b3t", (128, 4), F32)
    b4t = din("b4t", (128, 1), F32)
    y = nc.dram_tensor("y", [128, 512], F32, kind="ExternalOutput")

    with tile.TileContext(nc) as tc:
        with tc.tile_pool(name="persist", bufs=1) as pp:

            def sbuf(name, shape, dt=F32):
                return pp.tile(list(shape), dt, name=name, tag=name)

            def load(name, dram, shape, dt=F32R):
                t = sbuf(name, shape, dt)
                nc.sync.dma_start(out=t[:], in_=dram[:])
                return t

            # ---- loads ordered by first use
            xqT_t = load("xqT", xqT, (128, 512))
            at_t = load("at", at, (128, 16 * 128))
            eoT_t = load("eoT", eoT, (128, 512))
            xbT_t = load("xbT", xbT, (128, 2048))
            wv_t = load("wv", wv_st, (D, 512))
            onesv_t = load("onesv", ones_v, (128, 256))
            w1p_t = [load(f"w1p{g}", w1p[g], (D, D)) for g in range(2)]
            w2p_t = [load(f"w2p{g}", w2p[g], (D, D)) for g in range(2)]
            w3_t = load("w3", w3, (D, 512))
            w4_t = load("w4", w4r, (128, 512))
            sel_t = load("sel", selt, (128, 128))
            onec_t = load("onec", ones_col, (128, 1))
            oner_t = load("oner", ones_row, (1, 128))
            b1_t = load("b1", b1t, (128, 1), F32)
            b2_t = load("b2", b2t, (128, 1), F32)
            b3_t = load("b3", b3t, (128, 4), F32)
            b4_t = load("b4", b4t, (128, 1), F32)

            gs = [sbuf(f"g{ah}", (128, 512), F32R) for ah in range(16)]
            vs = [sbuf(f"v{t}", (128, 512), F32R) for t in range(KT)]

            # ---------------- setup: G = A_h @ x_q^T, V = x_kv @ wv (packed)
            with tc.tile_pool(name="pset", bufs=2, space="PSUM") as pset:
                for ah in range(16):  # attn*8 + head
                    xsrc = xqT_t if ah < 8 else eoT_t
                    gp = pset.tile([128, 512], F32, name=f"gp{ah}", tag="gp")
                    nc.tensor.matmul(
                        gp[:],
                        lhsT=at_t[:, 128 * ah : 128 * (ah + 1)],
                        rhs=xsrc[:],
                        start=True,
                        stop=True,
                    )
                    nc.vector.tensor_copy(out=gs[ah][:], in_=gp[:])
                for t in range(KT):
                    vp = pset.tile([128, 512], F32, name=f"vp{t}", tag="vp")
                    nc.tensor.matmul(
                        vp[:],
                        lhsT=xbT_t[:, 128 * t : 128 * (t + 1)],
                        rhs=wv_t[:],
                        start=True,
                        stop=True,
                    )
                    nc.vector.tensor_copy(out=vs[t][:], in_=vp[:])
                    # denominator ones columns (after the V copy so they are
                    # not overwritten by the zero-padded projection)
                    nc.sync.dma_start(
                        out=vs[t][:].rearrange("p (c x) -> p c x", x=32)[:, :, 16],
                        in_=onesv_t[:, 16 * t : 16 * (t + 1)],
                    )

            # ---------------- attention
            acc_sb = [[sbuf(f"acc{a}{g}", (128, 512), F32R) for g in range(2)]
                      for a in range(2)]

            def attention(a, pa, ebp):
                """per head: 8 key-tile-pair steps of [2 score mms ->
                exp([128,1024], f32r out) -> 2 pv mms]; 4 heads accumulate
                into one PSUM bank at 32-partition offsets, drained per
                group of 4 heads."""
                for h in range(H):
                    ah = 8 * a + h
                    g, hh = h // 4, h % 4
                    c0 = 256 * a + 128 * g + 32 * hh
                    pv = pa.tile([32, 512], F32, name=f"pv{a}{h}",
                                 tag=f"pv{h % 2}")
                    for p in range(8):
                        sc = pa.tile([128, 1024], F32, bufs=2,
                                     name=f"sc{ah}{p}", tag="sc")
                        for i in range(2):
                            nc.tensor.matmul(
                                sc[:, 512 * i : 512 * (i + 1)],
                                lhsT=xbT_t[
                                    :, 128 * (2 * p + i) : 128 * (2 * p + i + 1)
                                ],
                                rhs=gs[ah][:],
                                start=True,
                                stop=True,
                            )
                        eb = ebp.tile([128, 1024], F32R, name="eb", tag="eb")
                        nc.scalar.activation(eb[:], sc[:], AF.Exp, scale=0.25)
                        for i in range(2):
                            nc.tensor.matmul(
                                pv[:],
                                lhsT=vs[2 * p + i][:, c0 : c0 + 32],
                                rhs=eb[:, 512 * i : 512 * (i + 1)],
                                start=(p == 0 and i == 0),
                                stop=(p == 7 and i == 1),
                                skip_group_check=True,
                            )
                    nc.vector.tensor_copy(
                        out=acc_sb[a][g][32 * hh : 32 * hh + 32, :],
                        in_=pv[:],
                    )

            def normalize_and_project(a, wp_t, pa):
                sa_n = []
                for g in range(2):
                    sbc = pa.tile([128, 512], F32, name=f"sbc{a}{g}",
                                  tag=f"nm{g}")
                    nc.tensor.matmul(sbc[:], lhsT=sel_t[:], rhs=acc_sb[a][g][:],
                                     start=True, stop=True)
                    rb = pp.tile([128, 512], F32, name=f"rb{a}{g}",
                                 tag=f"rb{g}")
                    nc.vector.reciprocal(out=rb[:], in_=sbc[:])
                    sn = pp.tile([128, 512], F32R, name=f"sn{a}{g}",
                                 tag=f"sn{g}")
                    nc.vector.tensor_mul(sn[:], acc_sb[a][g][:], rb[:])
                    sa_n.append(sn)
                rp = pa.tile([128, 512], F32, name=f"rp{a}", tag="nm0")
                for g in range(2):
                    nc.tensor.matmul(rp[:], lhsT=wp_t[g][:], rhs=sa_n[g][:],
                                     start=(g == 0), stop=(g == 1))
                return rp

            with tc.tile_pool(name="pattn", bufs=1, space="PSUM") as pa, \
                 tc.tile_pool(name="ebp", bufs=3) as ebp:
                attention(0, pa, ebp)
                rp1 = normalize_and_project(0, w1p_t, pa)
                r1T = sbuf("r1T", (128, 512))
                nc.vector.tensor_add(r1T[:], rp1[:], xqT_t[:])
                nc.vector.tensor_scalar_add(r1T[:], r1T[:], b1_t[:])

                attention(1, pa, ebp)
                rp2 = normalize_and_project(1, w2p_t, pa)
                r2T = sbuf("r2T", (128, 512), F32R)
                nc.vector.tensor_add(r2T[:], rp2[:], r1T[:])
                nc.vector.tensor_scalar_add(r2T[:], r2T[:], b2_t[:])

            # ---------------- layernorm (x - m) / var, var unbiased
            with tc.tile_pool(name="ptail", bufs=1, space="PSUM") as pt:
                sq = sbuf("sq", (128, 512), F32R)
                nc.vector.tensor_mul(sq[:], r2T[:], r2T[:])
                mp = pt.tile([1, 512], F32, name="mp", tag="st0")
                nc.tensor.matmul(mp[:], lhsT=onec_t[:], rhs=r2T[:],
                                 start=True, stop=True)
                sp = pt.tile([1, 512], F32, name="sp", tag="st1")
                nc.tensor.matmul(sp[:], lhsT=onec_t[:], rhs=sq[:],
                                 start=True, stop=True)
                msb = sbuf("msb", (1, 512))
                nc.vector.tensor_copy(out=msb[:], in_=mp[:])
                ssb = sbuf("ssb", (1, 512))
                nc.vector.tensor_copy(out=ssb[:], in_=sp[:])
                t0 = sbuf("t0", (1, 512))
                nc.vector.tensor_mul(t0[:], msb[:], msb[:])
                nc.vector.tensor_scalar_mul(t0[:], t0[:], 1.0 / 128)
                nc.vector.tensor_sub(t0[:], ssb[:], t0[:])  # sum((x-m)^2)
                asb = sbuf("asb", (1, 512), F32R)
                with nc.allow_low_precision(reason="f32r is full fp32"):
                    nc.vector.reciprocal(out=asb[:], in_=t0[:])
                nc.vector.tensor_scalar_mul(asb[:], asb[:], 127.0)  # 1/var
                bsb = sbuf("bsb", (1, 512), F32R)
                nc.vector.tensor_mul(bsb[:], msb[:], asb[:])
                nc.vector.tensor_scalar_mul(bsb[:], bsb[:], -1.0 / 128)
                abc = pt.tile([128, 512], F32, name="abc", tag="bc0")
                nc.tensor.matmul(abc[:], lhsT=oner_t[:], rhs=asb[:],
                                 start=True, stop=True)
                bbc = pt.tile([128, 512], F32, name="bbc", tag="bc1")
                nc.tensor.matmul(bbc[:], lhsT=oner_t[:], rhs=bsb[:],
                                 start=True, stop=True)
                lnT = sbuf("lnT", (128, 512), F32R)
                nc.vector.tensor_mul(lnT[:], r2T[:], abc[:])
                nc.vector.tensor_add(lnT[:], lnT[:], bbc[:])

                # ---------------- FFN
                h_sb = []
                for j in range(4):
                    hp = pt.tile([128, 512], F32, name=f"hp{j}", tag=f"hp{j % 2}")
                    nc.tensor.matmul(hp[:],
                                     lhsT=w3_t[:, 128 * j : 128 * (j + 1)],
                                     rhs=lnT[:], start=True, stop=True)
                    hs = sbuf(f"hs{j}", (128, 512), F32R)
                    nc.vector.tensor_scalar(
                        out=hs[:], in0=hp[:], scalar1=b3_t[:, j : j + 1],
                        scalar2=0.0, op0=OP.add, op1=OP.max,
                    )
                    h_sb.append(hs)
                op_ = pt.tile([128, 512], F32, name="op", tag="bc0")
                for j in range(4):
                    nc.tensor.matmul(op_[:],
                                     lhsT=w4_t[:, 128 * j : 128 * (j + 1)],
                                     rhs=h_sb[j][:],
                                     start=(j == 0), stop=(j == 3),
                                     skip_group_check=True)
                oT = sbuf("oT", (128, 512))
                nc.vector.tensor_add(oT[:], op_[:], r2T[:])
                nc.vector.tensor_scalar_add(oT[:], oT[:], b4_t[:])
                nc.sync.dma_start(out=y[:], in_=oT[:])

    return nc


_CACHED = {}


def _get_nc():
    if "nc" not in _CACHED:
        _CACHED["nc"] = _split_multiwaits(build_nc())
    return _CACHED["nc"]


def _host_inputs(x_tgt, enc_out, self_wq, self_wk, self_wv, cross_wq, cross_wk,
                 cross_wv, w1, b1, w2, b2, w3, b3, w4, b4):
    # folded score matrices A_h = wk_h @ wq_h^T (contract head_dim on host)
    at = np.zeros((2, H, D, D), np.float32)
    for a, (wq, wk) in enumerate(((self_wq, self_wk), (cross_wq, cross_wk))):
        for h in range(H):
            at[a, h] = (wk[:, h::H] @ wq[:, h::H].T).T
    at_flat = np.ascontiguousarray(
        at.reshape(16, D, D).transpose(1, 0, 2).reshape(D, 16 * D)
    )

    shared = {
        "at": at_flat,
        "wv_st": np.concatenate(
            [_pack32_cols(self_wv, 0), _pack32_cols(self_wv, 1),
             _pack32_cols(cross_wv, 0), _pack32_cols(cross_wv, 1)], axis=1
        ),
        "ones_v": np.ones((128, 256), np.float32),
        "w1p0": _pack_w12(w1, 0), "w1p1": _pack_w12(w1, 1),
        "w2p0": _pack_w12(w2, 0), "w2p1": _pack_w12(w2, 1),
        "w3": w3,
        "w4r": np.ascontiguousarray(
            w4.reshape(4, 128, 128).transpose(1, 0, 2).reshape(128, 512)
        ),
        "selt": _sel_matrix(),
        "ones_col": np.ones((128, 1), np.float32),
        "ones_row": np.ones((1, 128), np.float32),
        "b1t": b1.reshape(128, 1),
        "b2t": b2.reshape(128, 1),
        "b3t": np.ascontiguousarray(b3.reshape(4, 128).T),
        "b4t": b4.reshape(128, 1),
    }
    shared = {k: np.ascontiguousarray(v, dtype=np.float32)
              for k, v in shared.items()}

    in_maps = []
    for c in range(NCORES):
        b, qb = divmod(c, 4)
        q0 = qb * QC
        im = dict(shared)
        im["xbT"] = np.ascontiguousarray(x_tgt[b].T)
        im["xqT"] = np.ascontiguousarray(x_tgt[b, q0 : q0 + QC].T)
        im["eoT"] = np.ascontiguousarray(enc_out[b, q0 : q0 + QC].T)
        in_maps.append(im)
    return in_maps


def run_on_device(in_maps, **kw):
    nc = _get_nc()
    return run_bass_kernel_spmd(nc, in_maps, list(range(NCORES)), **kw)


def kernel(x_tgt, enc_out, self_wq, self_wk, self_wv, cross_wq, cross_wk,
           cross_wv, w1, b1, w2, b2, w3, b3, w4, b4, mask_src=None,
           mask_tgt=None, **_unused):
    args = [x_tgt, enc_out, self_wq, self_wk, self_wv, cross_wq, cross_wk,
            cross_wv, w1, b1, w2, b2, w3, b3, w4, b4]
    args = [np.asarray(a, dtype=np.float32) for a in args]
    in_maps = _host_inputs(*args)
    res = run_on_device(in_maps)
    out = np.empty((B, S, D), np.float32)
    for c in range(NCORES):
        b, qb = divmod(c, 4)
        out[b, qb * QC : (qb + 1) * QC] = res.results[c]["y"].T
    return out
